# revision 1
# baseline (speedup 1.0000x reference)
"""Trainium2 Bass kernel for ARM TTT multi-head self-attention (inner-GD scan).

Math per (b, h) pair (B=16, H=12, N=4096, D=64, 16 chunks of m=256 tokens):
    A_i = k_i^T k_i ;  ct_i = k_i^T (-v_i)      (token contraction)
    grad_raw_i = A_i @ W_{i-1} + ct_i
    W_i = W_{i-1} - s * grad_raw_i,  s = 1/(m*D)
    out_i = q_i @ W_i
Pairs are fully independent -> shard B over the 8 NeuronCores (24 chains/core).

Two heads are packed per chain ("pair 0" on partitions 0:64, "pair 1" on
64:128).  k/v ship as fp8-e4m3 (the 256-token contraction averages out the
quantization); q ships bf16.

Measured-bottleneck-driven structure (v5):
  - The W-chain's critical path is the per-chunk PE->DVE->PE round trip, so
    the chunk update is ONE 128-partition DVE op: both heads' grad_raw land
    in a single scratch PSUM bank, seeded with ct by a full-width identity
    matmul (Id^T @ ct), then the two corner grad matmuls accumulate A@W on
    top.  The full-width Id matmul also serializes the two corner matmuls
    on the array, so their same-bank drains can never overlap (same-bank
    concurrent drains are a hardware-fatal NRT_EXEC_UNIT error - probed).
    An explicit ordering dep pins G1 after Id(next) against scheduler
    reordering.
  - [A|ct] matmuls are col-tiled per pair into separate banks and cast
    wholesale to SBUF (A and ct) right away, freeing the PSUM bank - this
    pays for the scratch-grad + 2-t-block-deep act->chain pipeline within
    the 8-bank budget.
  - q@W matmuls are row-tiled per pair into separate banks and delayed by
    one chunk so they never wait on the chain round trip.
  - One rolling pipeline over (group, t-block) items: acts of item n are
    emitted with the chain of item n-2, which both hides the chain latency
    and keeps casts two t-blocks ahead of the chain's Id reads.

Device layouts (token t = c*256 + 2p + j, chunk c, partition p, parity j):
    kv (per head):      (128, 16, 2, 2, 64)  [p, c, j, k|v, d]      fp8
    qt (per head-pair): (128, 16, 2, 128)    [pair*64+d, c, j, p]   bf16
    out (per pair):     (128, 16, 2, 2, 64)  [p, c, j, pair, e]     bf16
    W12: (128, 64) f32 init, rows 0:64 = head 2g, 64:128 = head 2g+1;
    the carried W chain itself is bf16.
"""

import os
import sys

sys.path.insert(0, "/opt/trn_rl_repo")

import numpy as np

B, H, N, D = 16, 12, 4096, 64
N_ITERS = 16
M = N // N_ITERS  # 256 tokens per chunk
NCORES = 8
NB = B // NCORES  # batches per core
HG = H // 2  # head-groups (pairs of heads) per batch
SCALE = 1.0 / (M * D)
CB = 4  # chunks per t-block

_CACHE = {}


def _split_excess_waits(nc):
    """walrus in this env accepts at most ONE sem wait per instruction
    (two on EventSemaphore); this snapshot's Tile wait-assigner attaches
    one wait per dependency proc directly to instructions.  Post-pass:
    move excess waits onto EventSemaphore instructions inserted just
    before the over-subscribed instruction on the same engine (engines
    execute their stream in order, so all waits still gate it)."""
    import concourse.mybir as mybir

    n_ev = 0
    for f in nc.m.functions:
        for b in f.blocks:
            il = b.instructions
            idx = 0
            while idx < len(il):
                inst = il[idx]
                si = getattr(inst, "sync_info", None)
                if si is not None and len(si.on_wait) > 1:
                    waits = list(si.on_wait)
                    si.on_wait = [waits[0]]
                    extra = waits[1:]
                    for g in range(0, len(extra), 2):
                        n_ev += 1
                        ev = mybir.InstEventSemaphore(
                            name=f"EVSPLIT-{n_ev}",
                            engine=inst.engine,
                            ins=[],
                            outs=[],
                            sync_info=mybir.SyncInfo(
                                on_wait=extra[g : g + 2], on_update=[]
                            ),
                        )
                        nc.register_instruction(ev)
                        il.insert(idx, ev)
                        idx += 1
                idx += 1
    return n_ev


class _G:
    """Per head-group (2 heads) on-device state."""

    __slots__ = ("kv1", "kv2", "qt", "outsb", "wrep", "pac0", "pac1",
                 "ac", "pout0", "pout1", "b", "gi")


def _build(nb=NB, hg=HG, n_iters=N_ITERS, cb=CB):
    import concourse.bass as bass
    import concourse.mybir as mybir
    from concourse.tile import TileContext
    from concourse.tile_rust import add_dep_helper

    f32 = mybir.dt.float32
    bf16 = mybir.dt.bfloat16
    fp8 = mybir.dt.float8e4
    Copy = mybir.ActivationFunctionType.Copy
    mult = mybir.AluOpType.mult
    add = mybir.AluOpType.add

    n_tb = n_iters // cb  # t-blocks per group
    ngroups = nb * hg

    nc = bass.Bass()
    q_d = nc.declare_dram_parameter(
        "qt", [nb, hg, 128, n_iters * 2 * 128], bf16, isOutput=False
    )
    kv_d = nc.declare_dram_parameter(
        "kv", [nb, 2 * hg, 128, n_iters * 2 * 2 * D], fp8, isOutput=False
    )
    w_d = nc.declare_dram_parameter("W12", [hg, 128, D], f32, isOutput=False)
    id_d = nc.declare_dram_parameter("ident", [128, 128], bf16, isOutput=False)
    out_d = nc.declare_dram_parameter(
        "out", [nb, hg, 128, n_iters * 2 * 2 * D], bf16, isOutput=True
    )

    with TileContext(nc) as tc:
        with (
            tc.tile_pool(name="singles", bufs=1) as singles,
            tc.tile_pool(name="kv", bufs=8) as kv_pool,
            tc.tile_pool(name="qt", bufs=5) as qt_pool,
            tc.tile_pool(name="osb", bufs=3) as osb_pool,
            tc.tile_pool(name="wr0", bufs=6) as wr0_pool,
            tc.tile_pool(name="wrp", bufs=8) as wrp_pool,
            tc.tile_pool(name="sm", bufs=5) as sm_pool,
            tc.tile_pool(name="pac", bufs=2, space="PSUM") as pac_pool,
            tc.tile_pool(name="pg", bufs=3, space="PSUM") as pg_pool,
            tc.tile_pool(name="pout", bufs=3, space="PSUM") as pout_pool,
        ):
            winit = singles.tile([128, hg, D], f32)
            nc.sync.dma_start(out=winit, in_=w_d.rearrange("g p e -> p g e"))
            ident = singles.tile([128, 128], bf16)
            nc.sync.dma_start(out=ident, in_=id_d[:, :])

            def make_group(gidx):
                g = _G()
                g.b, g.gi = divmod(gidx, hg)
                g.kv1 = kv_pool.tile([128, n_iters, 2, 2, D], fp8, tag="kv")
                g.kv2 = kv_pool.tile([128, n_iters, 2, 2, D], fp8, tag="kv")
                nc.sync.dma_start(
                    out=g.kv1,
                    in_=kv_d[g.b, 2 * g.gi].rearrange(
                        "p (c j s d) -> p c j s d", j=2, s=2, d=D
                    ),
                )
                nc.sync.dma_start(
                    out=g.kv2,
                    in_=kv_d[g.b, 2 * g.gi + 1].rearrange(
                        "p (c j s d) -> p c j s d", j=2, s=2, d=D
                    ),
                )
                g.qt = qt_pool.tile([128, n_iters, 2, 128], bf16, tag="qt")
                nc.sync.dma_start(
                    out=g.qt,
                    in_=q_d[g.b, g.gi].rearrange(
                        "p (c j t) -> p c j t", j=2, t=128
                    ),
                )
                g.wrep = wr0_pool.tile([128, D], bf16, tag="wrep0")
                nc.vector.tensor_copy(g.wrep, winit[:, g.gi, :])
                g.ac = {}
                g.outsb = None
                return g

            def emit_act_chunk(g, t, u):
                # [A|ct] matmuls, col-tiled pair into SEPARATE banks:
                # pair0 -> pac0[0:64, u, :], pair1 -> pac1[64:128, u, :]
                if u == 0:
                    g.pac0 = pac_pool.tile([128, cb, 128], f32, tag="pac")
                    g.pac1 = pac_pool.tile([128, cb, 128], f32, tag="pac")
                c = t * cb + u
                for j in (0, 1):
                    nc.tensor.matmul(
                        g.pac0[0:64, u, :],
                        lhsT=g.kv1[:, c, j, 0, :],
                        rhs=g.kv1[:, c, j, :, :],
                        start=(u == 0 and j == 0),
                        stop=(u == cb - 1 and j == 1),
                        skip_group_check=True,
                    )
                    nc.tensor.matmul(
                        g.pac1[64:128, u, :],
                        lhsT=g.kv2[:, c, j, 0, :],
                        rhs=g.kv2[:, c, j, :, :],
                        start=(u == 0 and j == 0),
                        stop=(u == cb - 1 and j == 1),
                        skip_group_check=True,
                    )

            def emit_cast(g, t):
                # evacuate [A|ct] of both pairs to SBUF bf16, freeing the
                # PSUM pair right away (the chain runs 2 t-blocks behind)
                ac = sm_pool.tile([128, cb, 128], bf16, tag="ac")
                nc.scalar.activation(
                    ac[0:64], g.pac0[0:64, :, :], func=Copy, scale=1.0
                )
                nc.scalar.activation(
                    ac[64:128], g.pac1[64:128, :, :], func=Copy, scale=1.0
                )
                g.ac[t] = ac

            # ---- chain bookkeeping -------------------------------------
            chain_q = []  # per chunk: [gobj, t, u, pgA, pgB]
            pos = [0]
            pending = [None]  # delayed q@W: (g, t, u, wrep)

            def emit_id(i):
                # seed the chunk's TWO scratch grad banks with ct via
                # col-split identity matmuls (concurrent, separate banks)
                g, t, u = chain_q[i][0], chain_q[i][1], chain_q[i][2]
                pga = pg_pool.tile([128, 512], f32, tag="pg")
                pgb = pg_pool.tile([128, 512], f32, tag="pg")
                nc.tensor.matmul(
                    pga[0:64, 0:64],
                    lhsT=ident[:, 0:64],
                    rhs=g.ac[t][:, u, 64:128],
                    start=True, stop=False, skip_group_check=True,
                )
                nc.tensor.matmul(
                    pgb[64:128, 0:64],
                    lhsT=ident[:, 64:128],
                    rhs=g.ac[t][:, u, 64:128],
                    start=True, stop=False, skip_group_check=True,
                )
                chain_q[i][3] = pga
                chain_q[i][4] = pgb

            def emit_out(g, t, u, wrep):
                # q @ W_i, row-tiled pair into SEPARATE banks
                if u == 0:
                    g.pout0 = pout_pool.tile([128, cb, 2, D], f32, tag="po")
                    g.pout1 = pout_pool.tile([128, cb, 2, D], f32, tag="po")
                c = t * cb + u
                for j in (0, 1):
                    nc.tensor.matmul(
                        g.pout0[:, u, j, :],
                        lhsT=g.qt[0:64, c, j, :],
                        rhs=wrep[0:64, :],
                        start=(u == 0 and j == 0),
                        stop=(u == cb - 1 and j == 1),
                        skip_group_check=True,
                    )
                    nc.tensor.matmul(
                        g.pout1[:, u, j, :],
                        lhsT=g.qt[64:128, c, j, :],
                        rhs=wrep[64:128, :],
                        start=(u == 0 and j == 0),
                        stop=(u == cb - 1 and j == 1),
                        skip_group_check=True,
                    )

            def flush_pending():
                if pending[0] is None:
                    return
                g, t, u, wrep = pending[0]
                pending[0] = None
                emit_out(g, t, u, wrep)
                if u == cb - 1:
                    if g.outsb is None:
                        g.outsb = osb_pool.tile(
                            [128, n_iters, 2, 2, D], bf16, tag="osb"
                        )
                    sl = slice(t * cb, (t + 1) * cb)
                    nc.scalar.copy(g.outsb[:, sl, :, 0, :], g.pout0)
                    nc.scalar.copy(g.outsb[:, sl, :, 1, :], g.pout1)
                    if t == n_tb - 1:
                        nc.scalar.dma_start(
                            out=out_d[g.b, g.gi], in_=g.outsb
                        )
                        del g.ac  # release t-block cast refs

            def chain_step():
                i = pos[0]
                e = chain_q[i]
                g, t, u = e[0], e[1], e[2]
                if e[3] is None:  # very first chunk: seed its own ct
                    emit_id(i)
                pga, pgb = e[3], e[4]
                # corner grad matmuls: concurrent, SEPARATE banks
                nc.tensor.matmul(
                    pga[0:64, 0:64],
                    lhsT=g.ac[t][0:64, u, 0:64],
                    rhs=g.wrep[0:64, :],
                    start=False, stop=True, skip_group_check=True,
                )
                nc.tensor.matmul(
                    pgb[64:128, 0:64],
                    lhsT=g.ac[t][64:128, u, 0:64],
                    rhs=g.wrep[64:128, :],
                    start=False, stop=True, skip_group_check=True,
                )
                if i + 1 < len(chain_q):
                    emit_id(i + 1)
                wrep = wrp_pool.tile([128, D], bf16, tag="wrep")
                nc.vector.scalar_tensor_tensor(
                    wrep[0:64], pga[0:64, 0:64], -SCALE,
                    g.wrep[0:64], op0=mult, op1=add,
                )
                nc.vector.scalar_tensor_tensor(
                    wrep[64:128], pgb[64:128, 0:64], -SCALE,
                    g.wrep[64:128], op0=mult, op1=add,
                )
                g.wrep = wrep
                flush_pending()
                pending[0] = (g, t, u, wrep)
                chain_q[i] = None  # drop refs
                pos[0] += 1

            # ---- rolling pipeline: acts(item n) + chain(item n-2) ------
            glist = [None] * (ngroups + 2)

            def ensure_group(i):
                if i < ngroups and glist[i] is None:
                    glist[i] = make_group(i)

            ensure_group(0)
            ensure_group(1)
            n_items = ngroups * n_tb
            for n in range(n_items + 2):
                ga_obj = None
                if n < n_items:
                    ga, ta = divmod(n, n_tb)
                    if ta == 0:
                        ensure_group(ga + 2)
                    ga_obj = glist[ga]
                for u in range(cb):
                    if n >= 2:
                        chain_step()
                    if ga_obj is not None:
                        emit_act_chunk(ga_obj, ta, u)
                if ga_obj is not None:
                    emit_cast(ga_obj, ta)
                    for u in range(cb):
                        chain_q.append([ga_obj, ta, u, None, None])
            flush_pending()

    _split_excess_waits(nc)
    return nc


def _get_nc():
    if "nc" not in _CACHE:
        _CACHE["nc"] = _build()
    return _CACHE["nc"]


def _host_prep(q, k, v):
    """Re-layout + pre-cast inputs on host (token t = c*256 + 2p + j)."""
    import ml_dtypes

    bf = ml_dtypes.bfloat16
    f8 = ml_dtypes.float8_e4m3
    Bq, Hq, Nq, Dq = q.shape
    ni = Nq // 256
    k6 = k.reshape(Bq, Hq, ni, 128, 2, Dq).transpose(0, 1, 3, 2, 4, 5)
    v6 = (-v).reshape(Bq, Hq, ni, 128, 2, Dq).transpose(0, 1, 3, 2, 4, 5)
    kv = np.ascontiguousarray(
        np.stack([k6, v6], axis=5).reshape(Bq, Hq, 128, ni * 2 * 2 * Dq)
    ).astype(f8)
    # qt: heads stacked pairwise on the partition dim: [b, hg, (pair d), (c j p)]
    q7 = q.reshape(Bq, Hq // 2, 2, ni, 128, 2, Dq)
    qt = np.ascontiguousarray(
        q7.transpose(0, 1, 2, 6, 3, 5, 4).reshape(Bq, Hq // 2, 128, ni * 2 * 128)
    ).astype(bf)
    return kv, qt


def _host_unshuffle(out_host):
    """(B, HG, 128, ni*2*2*64) bf16 [b,hg,p,(c,j,pair,e)] -> (B, N, H*64)."""
    Bq, hgq, _, w = out_host.shape
    ni = w // (2 * 2 * 64)
    o7 = np.asarray(out_host, dtype=np.float32).reshape(
        Bq, hgq, 128, ni, 2, 2, 64
    )
    # [b,hg,p,c,j,pair,e] -> [b,c,p,j,hg,pair,e]
    return np.ascontiguousarray(
        o7.transpose(0, 3, 2, 4, 1, 5, 6).reshape(Bq, ni * 256, hgq * 2 * 64)
    )


def kernel(q, k, v, W_init, training=0, return_aux=0, **_unused):
    import ml_dtypes
    from concourse.bass_utils import run_bass_kernel_spmd

    q = np.asarray(q, dtype=np.float32)
    k = np.asarray(k, dtype=np.float32)
    v = np.asarray(v, dtype=np.float32)
    W_init = np.ascontiguousarray(np.asarray(W_init, dtype=np.float32))

    kv, qt = _host_prep(q, k, v)
    W12 = np.ascontiguousarray(W_init.reshape(HG, 128, D))
    ident = np.eye(128, dtype=ml_dtypes.bfloat16)

    nc = _get_nc()
    in_maps = []
    for i in range(NCORES):
        sl = slice(i * NB, (i + 1) * NB)
        in_maps.append(
            {"qt": qt[sl], "kv": kv[sl], "W12": W12, "ident": ident}
        )

    trace = bool(int(os.environ.get("BASS_KERNEL_TRACE", "0")))
    res = run_bass_kernel_spmd(
        nc, in_maps, core_ids=list(range(NCORES)), trace=trace
    )
    _CACHE["last_results"] = res
    out_host = np.concatenate(
        [np.asarray(res.results[i]["out"]) for i in range(NCORES)], axis=0
    )
    return _host_unshuffle(out_host)


if __name__ == "__main__":
    rng = np.random.default_rng(0)
    q = rng.standard_normal((B, H, N, D), dtype=np.float32)
    k = rng.standard_normal((B, H, N, D), dtype=np.float32)
    v = rng.standard_normal((B, H, N, D), dtype=np.float32)
    W = (rng.standard_normal((H, D, D)) * D**-0.5).astype(np.float32)
    out = kernel(q, k, v, W)
    print("kernel ran, out shape:", out.shape)



# revision 6
# speedup vs baseline: 1.5574x; 1.5574x over previous
"""Trainium2 Bass kernel for ARM TTT multi-head self-attention (inner-GD scan).

Math per (b, h) pair (B=16, H=12, N=4096, D=64, 16 chunks of m=256 tokens):
    A_i = k_i^T k_i ;  ct_i = k_i^T (-v_i)      (token contraction)
    grad_raw_i = A_i @ W_{i-1} + ct_i
    W_i = W_{i-1} - s * grad_raw_i,  s = 1/(m*D)
    out_i = q_i @ W_i
Pairs are fully independent -> shard B over the 8 NeuronCores (24 chains/core).

v6: v5's measured bottleneck was PE instruction CADENCE (~116ns per matmul
regardless of size: LdWeights + dispatch), 12 matmuls per chunk.  v6 packs
each head-PAIR into block-diagonal 128x128 operands -> 4 matmuls per chunk:

  1. act:  ONE fp8 DoubleRow matmul per chunk contracts all 256 tokens:
           lhsT = [k0|k1] (128t x 2j x 128), rhs = [k0|k1|v0|v1] (x 256)
           -> pac[128, 256]: A0/A1 diag blocks of cols 0:128, ct0/ct1 diag
              blocks of cols 128:256 (junk off-diag).  4 chunks per pac.
  2. cast: per t-block (4 chunks), per pair, ONE activation moves the A/ct
           diag blocks into PERSISTENT pre-zeroed block-diag bf16 tiles
           (abct) - zeros off the diag keep the chain closed in block-diag.
  3. seed: matmul(pg = Id^T @ ctbd)   [start of PSUM accumulation group]
  4. grad: matmul(pg += Abd^T @ Wbd)  [stop]
     stt (DVE, ONE op): Wbd' = -s*pg + Wbd   (off-diag stays 0: 0*s+0)
  5. out:  matmul(pout = Wbd'(lhsT) @ qt[128 dpair x 256 tok]) - both heads
           in one 256-col matmul; emitted in chunk-PAIRS, one evac per pair.

The serial W-chain round trip (PE->DVE->PE ~1us) is hidden by round-robining
chunks across a 4-group window; acts/casts lead by LAG slots; outs trail.
PSUM budget (8 banks): pac 2x2 + pg 2 + pout 2.
GpSimd CANNOT access PSUM on TRN2, so casts go to Act and evacs to DVE/Act.

Device layouts (token t = c*256 + j*128 + p):
    kv (per group):  [128(p), 16(c), 2(j), 4(k0|k1|v0|v1), 64]  fp8 (v negated)
    qt (per group):  [128(hpair*64+d), 16(c), 256(t=j*128+p)]   bf16
    out (per group): [128(hpair*64+e), 16(c), 256(t)]           bf16
    W12bd: [hg, 128, 128] f32 block-diag(W_h0, W_h1); carried chain is bf16.
"""

import os
import sys

sys.path.insert(0, "/opt/trn_rl_repo")

import numpy as np

B, H, N, D = 16, 12, 4096, 64
N_ITERS = 16
M = N // N_ITERS  # 256 tokens per chunk
NCORES = 8
NB = B // NCORES  # batches per core
HG = H // 2  # head-pair groups per batch
SCALE = 1.0 / (M * D)
WAVE = 4  # chain interleave width (groups round-robined per chunk)
LAG = 8  # slots the act/cast stream leads the chain stream
CB = 4  # chunks per t-block (pac granularity)
USE_DR = True  # fp8 DoubleRow: one act matmul per chunk (else 2, j-accum)

_CACHE = {}


def _split_excess_waits(nc):
    """walrus in this env accepts at most ONE sem wait per instruction
    (two on EventSemaphore); move excess waits onto EventSemaphore
    instructions inserted just before on the same engine."""
    import concourse.mybir as mybir

    n_ev = 0
    for f in nc.m.functions:
        for b in f.blocks:
            il = b.instructions
            idx = 0
            while idx < len(il):
                inst = il[idx]
                si = getattr(inst, "sync_info", None)
                if si is not None and len(si.on_wait) > 1:
                    waits = list(si.on_wait)
                    si.on_wait = [waits[0]]
                    extra = waits[1:]
                    for g in range(0, len(extra), 2):
                        n_ev += 1
                        ev = mybir.InstEventSemaphore(
                            name=f"EVSPLIT-{n_ev}",
                            engine=inst.engine,
                            ins=[],
                            outs=[],
                            sync_info=mybir.SyncInfo(
                                on_wait=extra[g : g + 2], on_update=[]
                            ),
                        )
                        nc.register_instruction(ev)
                        il.insert(idx, ev)
                        idx += 1
                idx += 1
    return n_ev


class _G:
    __slots__ = ("kv", "qt", "outsb", "wrep", "abct", "pac", "b", "gi")


def _build(nb=NB, hg=HG, n_iters=N_ITERS):
    import concourse.bass as bass
    import concourse.mybir as mybir
    from concourse.tile import TileContext

    f32 = mybir.dt.float32
    bf16 = mybir.dt.bfloat16
    fp8 = mybir.dt.float8e4
    Copy = mybir.ActivationFunctionType.Copy
    mult = mybir.AluOpType.mult
    add = mybir.AluOpType.add
    DR = mybir.MatmulPerfMode.DoubleRow

    ngroups = nb * hg  # 12
    nwaves = ngroups // WAVE  # 3
    slots_per_wave = WAVE * n_iters  # 64
    n_tb = n_iters // CB  # 4 t-blocks per group

    nc = bass.Bass()
    q_d = nc.declare_dram_parameter(
        "qt", [nb, hg, 128, n_iters * 256], bf16, isOutput=False
    )
    kv_d = nc.declare_dram_parameter(
        "kv", [nb, hg, 128, n_iters * 2 * 4 * D], fp8, isOutput=False
    )
    w_d = nc.declare_dram_parameter("W12bd", [hg, 128, 128], f32, isOutput=False)
    id_d = nc.declare_dram_parameter("ident", [128, 128], bf16, isOutput=False)
    out_d = nc.declare_dram_parameter(
        "out", [nb, hg, 128, n_iters * 256], bf16, isOutput=True
    )

    with TileContext(nc) as tc:
        with (
            tc.tile_pool(name="singles", bufs=1) as singles,
            tc.tile_pool(name="kv", bufs=8) as kv_pool,
            tc.tile_pool(name="qt", bufs=7) as qt_pool,
            tc.tile_pool(name="osb", bufs=5) as osb_pool,
            tc.tile_pool(name="abct", bufs=6) as abct_pool,
            tc.tile_pool(name="wrp", bufs=14) as wrp_pool,
            tc.tile_pool(name="pac", bufs=2, space="PSUM") as pac_pool,
            tc.tile_pool(name="pg", bufs=2, space="PSUM") as pg_pool,
            tc.tile_pool(name="pout", bufs=2, space="PSUM") as pout_pool,
        ):
            winit = singles.tile([128, hg, 128], f32)
            nc.sync.dma_start(out=winit, in_=w_d.rearrange("g p e -> p g e"))
            ident = singles.tile([128, 128], bf16)
            nc.sync.dma_start(out=ident, in_=id_d[:, :])

            # persistent abct rotation: casts only ever write the diag
            # blocks, so the one-time memset zeros persist across reuses
            # (same logical tensors, manual rotation).
            abct_tiles = []
            for _ in range(6):
                t = abct_pool.tile([128, CB, 2, 128], bf16, tag="abct")
                nc.gpsimd.memset(t, 0.0)
                abct_tiles.append(t)
            abct_ctr = [0]

            glist = [None] * ngroups

            def ensure_group(gidx):
                if glist[gidx] is not None:
                    return
                g = _G()
                g.b, g.gi = divmod(gidx, hg)
                g.kv = kv_pool.tile([128, n_iters, 2, 4, D], fp8, tag="kv")
                nc.sync.dma_start(
                    out=g.kv,
                    in_=kv_d[g.b, g.gi].rearrange(
                        "p (c j s d) -> p c j s d", j=2, s=4, d=D
                    ),
                )
                g.wrep = wrp_pool.tile([128, 128], bf16, tag="wrep")
                nc.vector.tensor_copy(g.wrep, winit[:, g.gi, :])
                g.abct = {}
                g.pac = None
                g.qt = None
                g.outsb = None
                glist[gidx] = g

            def ensure_qt(gidx):
                g = glist[gidx]
                if g.qt is None:
                    g.qt = qt_pool.tile([128, n_iters, 256], bf16, tag="qt")
                    nc.sync.dma_start(
                        out=g.qt,
                        in_=q_d[g.b, g.gi].rearrange(
                            "p (c t) -> p c t", t=256
                        ),
                    )

            def emit_act(gidx, tb, u):
                # chunk c = CB*tb + u Gram matmul into pac[:, u, :, :]
                g = glist[gidx]
                c = CB * tb + u
                if u == 0:
                    g.pac = pac_pool.tile([128, CB, 2, 128], f32, tag="pac")
                if USE_DR:
                    nc.tensor.matmul(
                        g.pac[:, u, :, :],
                        lhsT=g.kv[:, c, :, 0:2, :],
                        rhs=g.kv[:, c, :, :, :],
                        start=True, stop=True,
                        perf_mode=DR,
                        skip_group_check=True,
                    )
                else:
                    for j in (0, 1):
                        nc.tensor.matmul(
                            g.pac[:, u, :, :],
                            lhsT=g.kv[:, c, j, 0:2, :],
                            rhs=g.kv[:, c, j, :, :],
                            start=(j == 0), stop=(j == 1),
                            skip_group_check=True,
                        )

            def emit_cast(gidx, tb):
                # A/ct diag blocks -> block-diag bf16 (abct off-diag stays 0)
                g = glist[gidx]
                ab = abct_tiles[abct_ctr[0] % len(abct_tiles)]
                abct_ctr[0] += 1
                nc.scalar.activation(
                    ab[0:64, :, :, 0:64], g.pac[0:64, :, :, 0:64],
                    func=Copy, scale=1.0,
                )
                nc.scalar.activation(
                    ab[64:128, :, :, 64:128], g.pac[64:128, :, :, 64:128],
                    func=Copy, scale=1.0,
                )
                g.abct[tb] = ab
                g.pac = None

            def chain_seed(g, c):
                tb, u = divmod(c, CB)
                ab = g.abct[tb]
                pg = pg_pool.tile([128, 512], f32, tag="pg")
                nc.tensor.matmul(
                    pg[:, 0:128],
                    lhsT=ident[:, :],
                    rhs=ab[:, u, 1, :],
                    start=True, stop=False, skip_group_check=True,
                )
                return pg

            def chain_grad(g, c, pg):
                tb, u = divmod(c, CB)
                ab = g.abct[tb]
                nc.tensor.matmul(
                    pg[:, 0:128],
                    lhsT=ab[:, u, 0, :],
                    rhs=g.wrep[:, :],
                    start=False, stop=True, skip_group_check=True,
                )
                wnew = wrp_pool.tile([128, 128], bf16, tag="wrep")
                nc.vector.scalar_tensor_tensor(
                    wnew, pg[:, 0:128], -SCALE, g.wrep,
                    op0=mult, op1=add,
                )
                g.wrep = wnew
                if u == CB - 1:
                    del g.abct[tb]

            def emit_out_pair(gidx, c0, w0, w1, evac_on_act):
                # outs for chunks c0, c0+1; called with a chain step between
                # the two matmuls handled by the caller via the 'mid' hook.
                g = glist[gidx]
                ensure_qt(gidx)
                po = pout_pool.tile([128, 2, 256], f32, tag="po")
                nc.tensor.matmul(
                    po[:, 0, :], lhsT=w0[:, :], rhs=g.qt[:, c0, :],
                    start=True, stop=True, skip_group_check=True,
                )
                return po

            def emit_out_pair2(gidx, c0, w1, po, evac_on_act):
                g = glist[gidx]
                nc.tensor.matmul(
                    po[:, 1, :], lhsT=w1[:, :], rhs=g.qt[:, c0 + 1, :],
                    start=True, stop=True, skip_group_check=True,
                )
                if g.outsb is None:
                    g.outsb = osb_pool.tile(
                        [128, n_iters, 256], bf16, tag="osb"
                    )
                dst = g.outsb[:, c0 : c0 + 2, :]
                if evac_on_act:
                    nc.scalar.activation(dst, po, func=Copy, scale=1.0)
                else:
                    nc.vector.tensor_copy(dst, po)
                if c0 + 1 == n_iters - 1:
                    nc.scalar.dma_start(out=out_d[g.b, g.gi], in_=g.outsb)
                    g.outsb = None
                    g.qt = None
                    g.kv = None

            # ---------------- schedule -----------------------------------
            # chain slot s (0..191): wave w = s//64, r = s%64, c = r//WAVE,
            #   gp = r%WAVE, group g = w*WAVE+gp.
            # act item (g, tb): 4 DR matmuls at slots w*64+16*tb+gp-LAG ...
            #   +3, cast at +4.
            # group kv DMA one wave ahead (spread), qt half a wave ahead.
            events = {}

            def at(slot, fn, *args):
                events.setdefault(slot, []).append((fn, args))

            n_slots = nwaves * slots_per_wave
            for w in range(nwaves):
                for gp in range(WAVE):
                    gidx = w * WAVE + gp
                    at(w * 64 - 64 + 16 * gp - LAG, ensure_group, gidx)
                    at(w * 64 - 28 + 6 * gp, ensure_qt, gidx)
                    for tb in range(n_tb):
                        t0 = w * 64 + 16 * tb + gp - LAG
                        for u in range(CB):
                            at(t0 + u, emit_act, gidx, tb, u)
                        at(t0 + CB, emit_cast, gidx, tb)

            # pending out-pairs: (gidx, c0, w0, w1)
            pend = []
            prev_w = [None] * ngroups
            evac_flip = [0]

            lo = min(events)
            for s in range(lo, n_slots + 3):
                for fn, args in events.get(s, ()):
                    fn(*args)
                # interleave within the slot so no two consecutive matmuls
                # target the same PSUM bank:
                #   out(c0) [pout] .. seed [pg] .. out(c0+1)+evac [pout]
                #   .. grad [pg]   (act matmuls land between slots)
                po_info = None
                if pend:
                    gq, c0q, w0q, w1q = pend.pop(0)
                    evac_flip[0] ^= 1
                    po = emit_out_pair(gq, c0q, w0q, w1q, evac_flip[0])
                    po_info = (gq, c0q, w1q, po, evac_flip[0])
                in_chain = 0 <= s < n_slots
                if in_chain:
                    w, r = divmod(s, slots_per_wave)
                    c, gp = divmod(r, WAVE)
                    gidx = w * WAVE + gp
                    g = glist[gidx]
                    pg = chain_seed(g, c)
                if po_info is not None:
                    emit_out_pair2(*po_info)
                if in_chain:
                    chain_grad(g, c, pg)
                    if c % 2 == 1:
                        pend.append((gidx, c - 1, prev_w[gidx], g.wrep))
                    else:
                        prev_w[gidx] = g.wrep

    _split_excess_waits(nc)
    return nc


def _get_nc():
    if "nc" not in _CACHE:
        _CACHE["nc"] = _build()
    return _CACHE["nc"]


def _host_prep(q, k, v):
    """Host re-layout (token t = c*256 + j*128 + p)."""
    import ml_dtypes

    bf = ml_dtypes.bfloat16
    f8 = ml_dtypes.float8_e4m3
    Bq, Hq, Nq, Dq = q.shape
    hg = Hq // 2
    ni = Nq // 256
    # kv: [b, g, p, c, j, (k0|k1|v0|v1), d]
    k7 = k.reshape(Bq, hg, 2, ni, 2, 128, Dq)
    v7 = (-v).reshape(Bq, hg, 2, ni, 2, 128, Dq)
    kv = np.stack(
        [k7[:, :, 0], k7[:, :, 1], v7[:, :, 0], v7[:, :, 1]], axis=5
    )  # [b, g, c, j, p, 4, d]
    kv = np.ascontiguousarray(
        kv.transpose(0, 1, 4, 2, 3, 5, 6).reshape(Bq, hg, 128, ni * 2 * 4 * Dq)
    ).astype(f8)
    # qt: [b, g, hpair*64+d, c, t]
    q6 = q.reshape(Bq, hg, 2, ni, 256, Dq)
    qt = np.ascontiguousarray(
        q6.transpose(0, 1, 2, 5, 3, 4).reshape(Bq, hg, 128, ni * 256)
    ).astype(bf)
    return kv, qt


def _host_unshuffle(out_host):
    """[b, g, hpair*64+e, c*256+t] bf16 -> (B, N, H*64) f32."""
    Bq, hgq, _, w = out_host.shape
    ni = w // 256
    o6 = np.asarray(out_host, dtype=np.float32).reshape(
        Bq, hgq, 2, 64, ni, 256
    )
    # [b, g, hp, e, c, t] -> [b, c, t, g, hp, e]
    return np.ascontiguousarray(
        o6.transpose(0, 4, 5, 1, 2, 3).reshape(Bq, ni * 256, hgq * 2 * 64)
    )


def kernel(q, k, v, W_init, training=0, return_aux=0, **_unused):
    import ml_dtypes
    from concourse.bass_utils import run_bass_kernel_spmd

    q = np.asarray(q, dtype=np.float32)
    k = np.asarray(k, dtype=np.float32)
    v = np.asarray(v, dtype=np.float32)
    W_init = np.ascontiguousarray(np.asarray(W_init, dtype=np.float32))

    kv, qt = _host_prep(q, k, v)
    wbd = np.zeros((HG, 128, 128), dtype=np.float32)
    wbd[:, 0:64, 0:64] = W_init[0::2]
    wbd[:, 64:128, 64:128] = W_init[1::2]
    ident = np.eye(128, dtype=ml_dtypes.bfloat16)

    nc = _get_nc()
    in_maps = []
    for i in range(NCORES):
        sl = slice(i * NB, (i + 1) * NB)
        in_maps.append(
            {"qt": qt[sl], "kv": kv[sl], "W12bd": wbd, "ident": ident}
        )

    trace = bool(int(os.environ.get("BASS_KERNEL_TRACE", "0")))
    res = run_bass_kernel_spmd(
        nc, in_maps, core_ids=list(range(NCORES)), trace=trace
    )
    _CACHE["last_results"] = res
    out_host = np.concatenate(
        [np.asarray(res.results[i]["out"]) for i in range(NCORES)], axis=0
    )
    return _host_unshuffle(out_host)


if __name__ == "__main__":
    rng = np.random.default_rng(0)
    q = rng.standard_normal((B, H, N, D), dtype=np.float32)
    k = rng.standard_normal((B, H, N, D), dtype=np.float32)
    v = rng.standard_normal((B, H, N, D), dtype=np.float32)
    W = (rng.standard_normal((H, D, D)) * D**-0.5).astype(np.float32)
    out = kernel(q, k, v, W)
    print("kernel ran, out shape:", out.shape)


# revision 16
# speedup vs baseline: 1.7569x; 1.1281x over previous
"""Trainium2 Bass kernel for ARM TTT multi-head self-attention (inner-GD scan).

Math per (b, h) pair (B=16, H=12, N=4096, D=64, 16 chunks of m=256 tokens):
    A_i = k_i^T k_i ;  ct_i = k_i^T (-v_i)      (token contraction)
    grad_raw_i = A_i @ W_{i-1} + ct_i
    W_i = W_{i-1} - s * grad_raw_i,  s = 1/(m*D)
    out_i = q_i @ W_i
Pairs are fully independent -> shard B over the 8 NeuronCores (24 chains/core).

v6: v5's measured bottleneck was PE instruction CADENCE (~116ns per matmul
regardless of size: LdWeights + dispatch), 12 matmuls per chunk.  v6 packs
each head-PAIR into block-diagonal 128x128 operands -> 4 matmuls per chunk:

  1. act:  ONE fp8 DoubleRow matmul per chunk contracts all 256 tokens:
           lhsT = [k0|k1] (128t x 2j x 128), rhs = [k0|k1|v0|v1] (x 256)
           -> pac[128, 256]: A0/A1 diag blocks of cols 0:128, ct0/ct1 diag
              blocks of cols 128:256 (junk off-diag).  4 chunks per pac.
  2. cast: per t-block (4 chunks), per pair, ONE activation moves the A/ct
           diag blocks into PERSISTENT pre-zeroed block-diag bf16 tiles
           (abct) - zeros off the diag keep the chain closed in block-diag.
  3. seed: matmul(pg = Id^T @ ctbd)   [start of PSUM accumulation group]
  4. grad: matmul(pg += Abd^T @ Wbd)  [stop]
     stt (DVE, ONE op): Wbd' = -s*pg + Wbd   (off-diag stays 0: 0*s+0)
  5. out:  matmul(pout = Wbd'(lhsT) @ qt[128 dpair x 256 tok]) - both heads
           in one 256-col matmul; emitted in chunk-PAIRS, one evac per pair.

The serial W-chain round trip (PE->DVE->PE ~1us) is hidden by round-robining
chunks across a 4-group window; acts/casts lead by LAG slots; outs trail.
PSUM budget (8 banks): pac 2x2 + pg 2 + pout 2.
GpSimd CANNOT access PSUM on TRN2, so casts go to Act and evacs to DVE/Act.

Device layouts (token t = c*256 + j*128 + p):
    kv (per group):  [128(p), 16(c), 2(j), 4(k0|k1|v0|v1), 64]  fp8 (v negated)
    qt (per group):  [128(hpair*64+d), 16(c), 256(t=j*128+p)]   bf16
    out (per group): [128(hpair*64+e), 16(c), 256(t)]           bf16
    W12bd: [hg, 128, 128] f32 block-diag(W_h0, W_h1); carried chain is bf16.
"""

import os
import sys

sys.path.insert(0, "/opt/trn_rl_repo")

import numpy as np

B, H, N, D = 16, 12, 4096, 64
N_ITERS = 16
M = N // N_ITERS  # 256 tokens per chunk
NCORES = 8
NB = B // NCORES  # batches per core
HG = H // 2  # head-pair groups per batch
SCALE = 1.0 / (M * D)
WAVE = 4  # chain interleave width (groups round-robined per chunk)
LAG = 14  # slots the act/cast stream leads the chain stream
CB = 4  # chunks per t-block (pac granularity)
USE_DR = True  # fp8 DoubleRow: one act matmul per chunk (else 2, j-accum)

_CACHE = {}


def _split_excess_waits(nc):
    """walrus in this env accepts at most ONE sem wait per instruction
    (two on EventSemaphore); move excess waits onto EventSemaphore
    instructions inserted just before on the same engine."""
    import concourse.mybir as mybir

    n_ev = 0
    for f in nc.m.functions:
        for b in f.blocks:
            il = b.instructions
            idx = 0
            while idx < len(il):
                inst = il[idx]
                si = getattr(inst, "sync_info", None)
                if si is not None and len(si.on_wait) > 1:
                    waits = list(si.on_wait)
                    si.on_wait = [waits[0]]
                    extra = waits[1:]
                    for g in range(0, len(extra), 2):
                        n_ev += 1
                        ev = mybir.InstEventSemaphore(
                            name=f"EVSPLIT-{n_ev}",
                            engine=inst.engine,
                            ins=[],
                            outs=[],
                            sync_info=mybir.SyncInfo(
                                on_wait=extra[g : g + 2], on_update=[]
                            ),
                        )
                        nc.register_instruction(ev)
                        il.insert(idx, ev)
                        idx += 1
                idx += 1
    return n_ev


class _G:
    __slots__ = ("kv", "qt", "outsb", "wrep", "abct", "pac", "b", "gi")


def _build(nb=NB, hg=HG, n_iters=N_ITERS):
    import concourse.bass as bass
    import concourse.mybir as mybir
    from concourse.tile import TileContext

    f32 = mybir.dt.float32
    bf16 = mybir.dt.bfloat16
    fp8 = mybir.dt.float8e4
    Copy = mybir.ActivationFunctionType.Copy
    mult = mybir.AluOpType.mult
    add = mybir.AluOpType.add
    DR = mybir.MatmulPerfMode.DoubleRow

    ngroups = nb * hg  # 12
    nwaves = ngroups // WAVE  # 3
    slots_per_wave = WAVE * n_iters  # 64
    n_tb = n_iters // CB  # 4 t-blocks per group

    nc = bass.Bass()
    q_d = nc.declare_dram_parameter(
        "qt", [nb, hg, 128, n_iters * 256], bf16, isOutput=False
    )
    kv_d = nc.declare_dram_parameter(
        "kv", [nb, hg, 128, n_iters * 2 * 4 * D], fp8, isOutput=False
    )
    w_d = nc.declare_dram_parameter("W12bd", [128, hg * 128], f32, isOutput=False)
    id_d = nc.declare_dram_parameter("ident", [128, 128], bf16, isOutput=False)
    out_d = nc.declare_dram_parameter(
        "out", [nb, hg, 128, n_iters * 256], bf16, isOutput=True
    )

    with TileContext(nc) as tc:
        with (
            tc.tile_pool(name="singles", bufs=1) as singles,
            tc.tile_pool(name="kv", bufs=8) as kv_pool,
            tc.tile_pool(name="qt", bufs=7) as qt_pool,
            tc.tile_pool(name="osb", bufs=5) as osb_pool,
            tc.tile_pool(name="abct", bufs=8) as abct_pool,
            tc.tile_pool(name="wrp", bufs=14) as wrp_pool,
            tc.tile_pool(name="pac", bufs=2, space="PSUM") as pac_pool,
            tc.tile_pool(name="pg", bufs=2, space="PSUM") as pg_pool,
            tc.tile_pool(name="pout", bufs=2, space="PSUM") as pout_pool,
        ):
            winit = singles.tile([128, hg, 128], f32)
            nc.sync.dma_start(
                out=winit, in_=w_d.rearrange("p (g e) -> p g e", g=hg)
            )
            ident = singles.tile([128, 128], bf16)
            nc.sync.dma_start(out=ident, in_=id_d[:, :])

            # persistent abct rotation: casts only ever write the diag
            # blocks, so the one-time memset zeros persist across reuses
            # (same logical tensors, manual rotation).
            abct_tiles = []
            for _ in range(8):
                t = abct_pool.tile([128, CB, 2, 128], bf16, tag="abct")
                nc.gpsimd.memset(t, 0.0)
                abct_tiles.append(t)
            abct_ctr = [0]

            glist = [None] * ngroups

            def ensure_group(gidx):
                if glist[gidx] is not None:
                    return
                g = _G()
                g.b, g.gi = divmod(gidx, hg)
                g.kv = kv_pool.tile([128, n_iters, 2, 4, D], fp8, tag="kv")
                nc.sync.dma_start(
                    out=g.kv,
                    in_=kv_d[g.b, g.gi].rearrange(
                        "p (c j s d) -> p c j s d", j=2, s=4, d=D
                    ),
                )
                g.wrep = wrp_pool.tile([128, 128], bf16, tag="wrep")
                nc.vector.tensor_copy(g.wrep, winit[:, g.gi, :])
                g.abct = {}
                g.pac = None
                g.qt = None
                g.outsb = None
                glist[gidx] = g

            def ensure_qt(gidx):
                g = glist[gidx]
                if g.qt is None:
                    g.qt = qt_pool.tile([128, n_iters, 256], bf16, tag="qt")
                    nc.sync.dma_start(
                        out=g.qt,
                        in_=q_d[g.b, g.gi].rearrange(
                            "p (c t) -> p c t", t=256
                        ),
                    )

            def emit_act(gidx, tb, u):
                # chunk c = CB*tb + u Gram matmul into pac[:, u, :, :]
                g = glist[gidx]
                c = CB * tb + u
                if u == 0:
                    g.pac = pac_pool.tile([128, CB, 2, 128], f32, tag="pac")
                if USE_DR:
                    nc.tensor.matmul(
                        g.pac[:, u, :, :],
                        lhsT=g.kv[:, c, :, 0:2, :],
                        rhs=g.kv[:, c, :, :, :],
                        start=True, stop=True,
                        perf_mode=DR,
                        skip_group_check=True,
                    )
                else:
                    for j in (0, 1):
                        nc.tensor.matmul(
                            g.pac[:, u, :, :],
                            lhsT=g.kv[:, c, j, 0:2, :],
                            rhs=g.kv[:, c, j, :, :],
                            start=(j == 0), stop=(j == 1),
                            skip_group_check=True,
                        )

            def emit_cast(gidx, tb):
                # A/ct diag blocks -> block-diag bf16 (abct off-diag stays 0)
                g = glist[gidx]
                ab = abct_tiles[abct_ctr[0] % len(abct_tiles)]
                on_dve = abct_ctr[0] % 4 == 3
                abct_ctr[0] += 1
                nc.scalar.activation(
                    ab[0:64, :, :, 0:64], g.pac[0:64, :, :, 0:64],
                    func=Copy, scale=1.0,
                )
                if on_dve:
                    nc.vector.tensor_copy(
                        ab[64:128, :, :, 64:128], g.pac[64:128, :, :, 64:128]
                    )
                else:
                    nc.scalar.activation(
                        ab[64:128, :, :, 64:128], g.pac[64:128, :, :, 64:128],
                        func=Copy, scale=1.0,
                    )
                g.abct[tb] = ab
                g.pac = None

            def chain_seed(g, c):
                tb, u = divmod(c, CB)
                ab = g.abct[tb]
                pg = pg_pool.tile([128, 512], f32, tag="pg")
                nc.tensor.matmul(
                    pg[:, 0:128],
                    lhsT=ident[:, :],
                    rhs=ab[:, u, 1, :],
                    start=True, stop=False, skip_group_check=True,
                )
                return pg

            def chain_grad(g, c, pg):
                tb, u = divmod(c, CB)
                ab = g.abct[tb]
                nc.tensor.matmul(
                    pg[:, 0:128],
                    lhsT=ab[:, u, 0, :],
                    rhs=g.wrep[:, :],
                    start=False, stop=True, skip_group_check=True,
                )
                wnew = wrp_pool.tile([128, 128], bf16, tag="wrep")
                nc.vector.scalar_tensor_tensor(
                    wnew, pg[:, 0:128], -SCALE, g.wrep,
                    op0=mult, op1=add,
                )
                g.wrep = wnew
                if u == CB - 1:
                    del g.abct[tb]

            def emit_out_pair(gidx, c0, w0, w1, evac_on_act):
                # outs for chunks c0, c0+1; called with a chain step between
                # the two matmuls handled by the caller via the 'mid' hook.
                g = glist[gidx]
                ensure_qt(gidx)
                po = pout_pool.tile([128, 2, 256], f32, tag="po")
                nc.tensor.matmul(
                    po[:, 0, :], lhsT=w0[:, :], rhs=g.qt[:, c0, :],
                    start=True, stop=True, skip_group_check=True,
                )
                return po

            def emit_out_pair2(gidx, c0, w1, po, evac_on_act):
                g = glist[gidx]
                nc.tensor.matmul(
                    po[:, 1, :], lhsT=w1[:, :], rhs=g.qt[:, c0 + 1, :],
                    start=True, stop=True, skip_group_check=True,
                )
                if g.outsb is None:
                    g.outsb = osb_pool.tile(
                        [128, n_iters, 256], bf16, tag="osb"
                    )
                dst = g.outsb[:, c0 : c0 + 2, :]
                if evac_on_act:
                    nc.scalar.activation(dst, po, func=Copy, scale=1.0)
                else:
                    nc.vector.tensor_copy(dst, po)
                half = n_iters * 256 // 2
                if c0 + 1 == n_iters // 2 - 1:
                    nc.gpsimd.dma_start(
                        out=out_d[g.b, g.gi, :, 0:half],
                        in_=g.outsb[:, 0 : n_iters // 2, :],
                    )
                elif c0 + 1 == n_iters - 1:
                    nc.gpsimd.dma_start(
                        out=out_d[g.b, g.gi, :, half : 2 * half],
                        in_=g.outsb[:, n_iters // 2 : n_iters, :],
                    )
                    g.outsb = None
                    g.qt = None
                    g.kv = None

            # ---------------- schedule -----------------------------------
            # chain slot s (0..191): wave w = s//64, r = s%64, c = r//WAVE,
            #   gp = r%WAVE, group g = w*WAVE+gp.
            # act item (g, tb): 4 DR matmuls at slots w*64+16*tb+gp-LAG ...
            #   +3, cast at +4.
            # group kv DMA one wave ahead (spread), qt half a wave ahead.
            events = {}

            def at(slot, fn, *args):
                events.setdefault(slot, []).append((fn, args))

            n_slots = nwaves * slots_per_wave
            for w in range(nwaves):
                for gp in range(WAVE):
                    gidx = w * WAVE + gp
                    at(w * 64 - 72 + 10 * gp, ensure_group, gidx)
                    at(w * 64 - 67 + 10 * gp, ensure_qt, gidx)
                    for tb in range(n_tb):
                        t0 = w * 64 + 16 * tb + 4 * gp - LAG
                        for u in range(CB):
                            at(t0 + u, emit_act, gidx, tb, u)
                        at(t0 + CB, emit_cast, gidx, tb)

            # pending out-pairs: (gidx, c0, w0, w1)
            pend = []
            prev_w = [None] * ngroups
            evac_flip = [0]

            lo = min(events)
            for s in range(lo, n_slots + 3):
                for fn, args in events.get(s, ()):
                    fn(*args)
                # interleave within the slot so no two consecutive matmuls
                # target the same PSUM bank:
                #   out(c0) [pout] .. seed [pg] .. out(c0+1)+evac [pout]
                #   .. grad [pg]   (act matmuls land between slots)
                po_info = None
                if pend:
                    gq, c0q, w0q, w1q = pend.pop(0)
                    evac_flip[0] = (evac_flip[0] + 1) % 3
                    on_act = evac_flip[0] != 0  # 2/3 Act, 1/3 DVE
                    po = emit_out_pair(gq, c0q, w0q, w1q, on_act)
                    po_info = (gq, c0q, w1q, po, on_act)
                in_chain = 0 <= s < n_slots
                if in_chain:
                    w, r = divmod(s, slots_per_wave)
                    c, gp = divmod(r, WAVE)
                    gidx = w * WAVE + gp
                    g = glist[gidx]
                    pg = chain_seed(g, c)
                if po_info is not None:
                    emit_out_pair2(*po_info)
                if in_chain:
                    chain_grad(g, c, pg)
                    if c % 2 == 1:
                        pend.append((gidx, c - 1, prev_w[gidx], g.wrep))
                    else:
                        prev_w[gidx] = g.wrep

    _split_excess_waits(nc)
    return nc


def _get_nc():
    if "nc" not in _CACHE:
        _CACHE["nc"] = _build()
    return _CACHE["nc"]


def _host_prep(q, k, v):
    """Host re-layout (token t = c*256 + j*128 + p)."""
    import ml_dtypes

    bf = ml_dtypes.bfloat16
    f8 = ml_dtypes.float8_e4m3
    Bq, Hq, Nq, Dq = q.shape
    hg = Hq // 2
    ni = Nq // 256
    # kv: [b, g, p, c, j, (k0|k1|v0|v1), d]
    k7 = k.reshape(Bq, hg, 2, ni, 2, 128, Dq)
    v7 = (-v).reshape(Bq, hg, 2, ni, 2, 128, Dq)
    kv = np.stack(
        [k7[:, :, 0], k7[:, :, 1], v7[:, :, 0], v7[:, :, 1]], axis=5
    )  # [b, g, c, j, p, 4, d]
    kv = np.ascontiguousarray(
        kv.transpose(0, 1, 4, 2, 3, 5, 6).reshape(Bq, hg, 128, ni * 2 * 4 * Dq)
    ).astype(f8)
    # qt: [b, g, hpair*64+d, c, t]
    q6 = q.reshape(Bq, hg, 2, ni, 256, Dq)
    qt = np.ascontiguousarray(
        q6.transpose(0, 1, 2, 5, 3, 4).reshape(Bq, hg, 128, ni * 256)
    ).astype(bf)
    return kv, qt


def _host_unshuffle(out_host):
    """[b, g, hpair*64+e, c*256+t] bf16 -> (B, N, H*64) f32."""
    Bq, hgq, _, w = out_host.shape
    ni = w // 256
    o6 = np.asarray(out_host, dtype=np.float32).reshape(
        Bq, hgq, 2, 64, ni, 256
    )
    # [b, g, hp, e, c, t] -> [b, c, t, g, hp, e]
    return np.ascontiguousarray(
        o6.transpose(0, 4, 5, 1, 2, 3).reshape(Bq, ni * 256, hgq * 2 * 64)
    )


def kernel(q, k, v, W_init, training=0, return_aux=0, **_unused):
    import ml_dtypes
    from concourse.bass_utils import run_bass_kernel_spmd

    q = np.asarray(q, dtype=np.float32)
    k = np.asarray(k, dtype=np.float32)
    v = np.asarray(v, dtype=np.float32)
    W_init = np.ascontiguousarray(np.asarray(W_init, dtype=np.float32))

    kv, qt = _host_prep(q, k, v)
    wbd = np.zeros((HG, 128, 128), dtype=np.float32)
    wbd[:, 0:64, 0:64] = W_init[0::2]
    wbd[:, 64:128, 64:128] = W_init[1::2]
    wbd = np.ascontiguousarray(
        wbd.transpose(1, 0, 2).reshape(128, HG * 128)
    )
    ident = np.eye(128, dtype=ml_dtypes.bfloat16)

    nc = _get_nc()
    in_maps = []
    for i in range(NCORES):
        sl = slice(i * NB, (i + 1) * NB)
        in_maps.append(
            {"qt": qt[sl], "kv": kv[sl], "W12bd": wbd, "ident": ident}
        )

    trace = bool(int(os.environ.get("BASS_KERNEL_TRACE", "0")))
    res = run_bass_kernel_spmd(
        nc, in_maps, core_ids=list(range(NCORES)), trace=trace
    )
    _CACHE["last_results"] = res
    out_host = np.concatenate(
        [np.asarray(res.results[i]["out"]) for i in range(NCORES)], axis=0
    )
    return _host_unshuffle(out_host)


if __name__ == "__main__":
    rng = np.random.default_rng(0)
    q = rng.standard_normal((B, H, N, D), dtype=np.float32)
    k = rng.standard_normal((B, H, N, D), dtype=np.float32)
    v = rng.standard_normal((B, H, N, D), dtype=np.float32)
    W = (rng.standard_normal((H, D, D)) * D**-0.5).astype(np.float32)
    out = kernel(q, k, v, W)
    print("kernel ran, out shape:", out.shape)


# revision 21
# speedup vs baseline: 1.8935x; 1.0778x over previous
"""Trainium2 Bass kernel for ARM TTT multi-head self-attention (inner-GD scan).

Math per (b, h) pair (B=16, H=12, N=4096, D=64, 16 chunks of m=256 tokens):
    A_i = k_i^T k_i ;  ct_i = k_i^T (-v_i)      (token contraction)
    grad_raw_i = A_i @ W_{i-1} + ct_i
    W_i = W_{i-1} - s * grad_raw_i,  s = 1/(m*D)
    out_i = q_i @ W_i
Pairs are fully independent -> shard B over the 8 NeuronCores (24 chains/core).

v6: v5's measured bottleneck was PE instruction CADENCE (~116ns per matmul
regardless of size: LdWeights + dispatch), 12 matmuls per chunk.  v6 packs
each head-PAIR into block-diagonal 128x128 operands -> 4 matmuls per chunk:

  1. act:  ONE fp8 DoubleRow matmul per chunk contracts all 256 tokens:
           lhsT = [k0|k1] (128t x 2j x 128), rhs = [k0|k1|v0|v1] (x 256)
           -> pac[128, 256]: A0/A1 diag blocks of cols 0:128, ct0/ct1 diag
              blocks of cols 128:256 (junk off-diag).  4 chunks per pac.
  2. cast: per t-block (4 chunks), per pair, ONE activation moves the A/ct
           diag blocks into PERSISTENT pre-zeroed block-diag bf16 tiles
           (abct) - zeros off the diag keep the chain closed in block-diag.
  3. seed: matmul(pg = Id^T @ ctbd)   [start of PSUM accumulation group]
  4. grad: matmul(pg += Abd^T @ Wbd)  [stop]
     stt (DVE, ONE op): Wbd' = -s*pg + Wbd   (off-diag stays 0: 0*s+0)
  5. out:  matmul(pout = Wbd'(lhsT) @ qt[128 dpair x 256 tok]) - both heads
           in one 256-col matmul; emitted in chunk-PAIRS, one evac per pair.

The serial W-chain round trip (PE->DVE->PE ~1us) is hidden by round-robining
chunks across a 4-group window; acts/casts lead by LAG slots; outs trail.
PSUM budget (8 banks): pac 2x2 + pg 2 + pout 2.
GpSimd CANNOT access PSUM on TRN2, so casts go to Act and evacs to DVE/Act.

Device layouts (token t = c*256 + j*128 + p):
    kv (per group):  [128(p), 16(c), 2(j), 4(k0|k1|v0|v1), 64]  fp8 (v negated)
    qt (per group):  [128(hpair*64+d), 16(c), 256(t=j*128+p)]   bf16
    out (per group): [128(hpair*64+e), 16(c), 256(t)]           bf16
    W12bd: [hg, 128, 128] f32 block-diag(W_h0, W_h1); carried chain is bf16.
"""

import os
import sys

sys.path.insert(0, "/opt/trn_rl_repo")

import numpy as np

B, H, N, D = 16, 12, 4096, 64
N_ITERS = 16
M = N // N_ITERS  # 256 tokens per chunk
NCORES = 8
NB = B // NCORES  # batches per core
HG = H // 2  # head-pair groups per batch
SCALE = 1.0 / (M * D)
WAVE = 4  # chain interleave width (groups round-robined per chunk)
LAG = 14  # slots the act/cast stream leads the chain stream
CB = 4  # chunks per t-block (pac granularity)
USE_DR = True  # fp8 DoubleRow: one act matmul per chunk (else 2, j-accum)

_CACHE = {}


def _split_excess_waits(nc):
    """walrus in this env accepts at most ONE sem wait per instruction
    (two on EventSemaphore); move excess waits onto EventSemaphore
    instructions inserted just before on the same engine."""
    import concourse.mybir as mybir

    n_ev = 0
    for f in nc.m.functions:
        for b in f.blocks:
            il = b.instructions
            idx = 0
            while idx < len(il):
                inst = il[idx]
                si = getattr(inst, "sync_info", None)
                if si is not None and len(si.on_wait) > 1:
                    waits = list(si.on_wait)
                    si.on_wait = [waits[0]]
                    extra = waits[1:]
                    for g in range(0, len(extra), 2):
                        n_ev += 1
                        ev = mybir.InstEventSemaphore(
                            name=f"EVSPLIT-{n_ev}",
                            engine=inst.engine,
                            ins=[],
                            outs=[],
                            sync_info=mybir.SyncInfo(
                                on_wait=extra[g : g + 2], on_update=[]
                            ),
                        )
                        nc.register_instruction(ev)
                        il.insert(idx, ev)
                        idx += 1
                idx += 1
    return n_ev


class _G:
    __slots__ = ("kv", "qt", "outsb", "wrep", "abct", "pac", "b", "gi")


def _build(nb=NB, hg=HG, n_iters=N_ITERS):
    import concourse.bass as bass
    import concourse.mybir as mybir
    from concourse.tile import TileContext

    f32 = mybir.dt.float32
    bf16 = mybir.dt.bfloat16
    fp8 = mybir.dt.float8e4
    Copy = mybir.ActivationFunctionType.Copy
    mult = mybir.AluOpType.mult
    add = mybir.AluOpType.add
    DR = mybir.MatmulPerfMode.DoubleRow

    ngroups = nb * hg  # 12
    nwaves = ngroups // WAVE  # 3
    slots_per_wave = WAVE * n_iters  # 64
    n_tb = n_iters // CB  # 4 t-blocks per group

    nc = bass.Bass()
    q_d = nc.declare_dram_parameter(
        "qt", [nb, hg, 128, n_iters * 256], bf16, isOutput=False
    )
    kv_d = nc.declare_dram_parameter(
        "kv", [nb, hg, 128, n_iters * 2 * 4 * D], fp8, isOutput=False
    )
    w_d = nc.declare_dram_parameter("W12bd", [128, hg * 128], f32, isOutput=False)
    id_d = nc.declare_dram_parameter("ident", [128, 128], bf16, isOutput=False)
    out_d = nc.declare_dram_parameter(
        "out", [nb, hg, 128, n_iters * 256], bf16, isOutput=True
    )

    with TileContext(nc) as tc:
        with (
            tc.tile_pool(name="singles", bufs=1) as singles,
            tc.tile_pool(name="kv", bufs=8) as kv_pool,
            tc.tile_pool(name="qt", bufs=7) as qt_pool,
            tc.tile_pool(name="osb", bufs=5) as osb_pool,
            tc.tile_pool(name="abct", bufs=8) as abct_pool,
            tc.tile_pool(name="wrp", bufs=14) as wrp_pool,
            tc.tile_pool(name="pac", bufs=2, space="PSUM") as pac_pool,
            tc.tile_pool(name="pg", bufs=2, space="PSUM") as pg_pool,
            tc.tile_pool(name="pout", bufs=2, space="PSUM") as pout_pool,
        ):
            winit = singles.tile([128, hg, 128], f32)
            nc.sync.dma_start(
                out=winit, in_=w_d.rearrange("p (g e) -> p g e", g=hg)
            )
            ident = singles.tile([128, 128], bf16)
            nc.sync.dma_start(out=ident, in_=id_d[:, :])

            # persistent abct rotation: casts only ever write the diag
            # blocks, so the one-time memset zeros persist across reuses
            # (same logical tensors, manual rotation).
            abct_tiles = []
            for _ in range(8):
                t = abct_pool.tile([128, CB, 2, 128], bf16, tag="abct")
                nc.gpsimd.memset(t, 0.0)
                abct_tiles.append(t)
            abct_ctr = [0]

            glist = [None] * ngroups

            def ensure_group(gidx):
                if glist[gidx] is not None:
                    return
                g = _G()
                g.b, g.gi = divmod(gidx, hg)
                g.wrep = wrp_pool.tile([128, 128], bf16, tag="wrep")
                nc.vector.tensor_copy(g.wrep, winit[:, g.gi, :])
                g.abct = {}
                g.pac = None
                g.kv = None
                g.qt = None
                g.outsb = None
                glist[gidx] = g

            def kv_half(gidx, h):
                # kv DMA in halves so acts can start after 0.5MB lands
                ensure_group(gidx)
                g = glist[gidx]
                if g.kv is None:
                    g.kv = kv_pool.tile(
                        [128, n_iters, 2, 4, D], fp8, tag="kv"
                    )
                hc = n_iters // 2
                w2 = hc * 2 * 4 * D
                nc.sync.dma_start(
                    out=g.kv[:, h * hc : (h + 1) * hc, :, :, :],
                    in_=kv_d[g.b, g.gi, :, h * w2 : (h + 1) * w2].rearrange(
                        "p (c j s d) -> p c j s d", j=2, s=4, d=D
                    ),
                )

            def qt_half(gidx, h):
                g = glist[gidx]
                if g.qt is None:
                    g.qt = qt_pool.tile([128, n_iters, 256], bf16, tag="qt")
                hc = n_iters // 2
                w2 = hc * 256
                nc.sync.dma_start(
                    out=g.qt[:, h * hc : (h + 1) * hc, :],
                    in_=q_d[g.b, g.gi, :, h * w2 : (h + 1) * w2].rearrange(
                        "p (c t) -> p c t", t=256
                    ),
                )

            def emit_act(gidx, tb, u):
                # chunk c = CB*tb + u Gram matmul into pac[:, u, :, :]
                g = glist[gidx]
                c = CB * tb + u
                if u == 0:
                    g.pac = pac_pool.tile([128, CB, 2, 128], f32, tag="pac")
                if USE_DR:
                    nc.tensor.matmul(
                        g.pac[:, u, :, :],
                        lhsT=g.kv[:, c, :, 0:2, :],
                        rhs=g.kv[:, c, :, :, :],
                        start=True, stop=True,
                        perf_mode=DR,
                        skip_group_check=True,
                    )
                else:
                    for j in (0, 1):
                        nc.tensor.matmul(
                            g.pac[:, u, :, :],
                            lhsT=g.kv[:, c, j, 0:2, :],
                            rhs=g.kv[:, c, j, :, :],
                            start=(j == 0), stop=(j == 1),
                            skip_group_check=True,
                        )

            def emit_cast(gidx, tb):
                # A/ct diag blocks -> block-diag bf16 (abct off-diag stays 0)
                g = glist[gidx]
                ab = abct_tiles[abct_ctr[0] % len(abct_tiles)]
                on_dve = abct_ctr[0] % 4 == 3
                abct_ctr[0] += 1
                nc.scalar.activation(
                    ab[0:64, :, :, 0:64], g.pac[0:64, :, :, 0:64],
                    func=Copy, scale=1.0,
                )
                if on_dve:
                    nc.vector.tensor_copy(
                        ab[64:128, :, :, 64:128], g.pac[64:128, :, :, 64:128]
                    )
                else:
                    nc.scalar.activation(
                        ab[64:128, :, :, 64:128], g.pac[64:128, :, :, 64:128],
                        func=Copy, scale=1.0,
                    )
                g.abct[tb] = ab
                g.pac = None

            def chain_seed(g, c):
                tb, u = divmod(c, CB)
                ab = g.abct[tb]
                pg = pg_pool.tile([128, 512], f32, tag="pg")
                nc.tensor.matmul(
                    pg[:, 0:128],
                    lhsT=ident[:, :],
                    rhs=ab[:, u, 1, :],
                    start=True, stop=False, skip_group_check=True,
                )
                return pg

            def chain_grad(g, c, pg):
                tb, u = divmod(c, CB)
                ab = g.abct[tb]
                nc.tensor.matmul(
                    pg[:, 0:128],
                    lhsT=ab[:, u, 0, :],
                    rhs=g.wrep[:, :],
                    start=False, stop=True, skip_group_check=True,
                )
                wnew = wrp_pool.tile([128, 128], bf16, tag="wrep")
                nc.vector.scalar_tensor_tensor(
                    wnew, pg[:, 0:128], -SCALE, g.wrep,
                    op0=mult, op1=add,
                )
                g.wrep = wnew
                if u == CB - 1:
                    del g.abct[tb]

            def emit_out_mm(gidx, c, wrep, po, slot_idx):
                g = glist[gidx]
                nc.tensor.matmul(
                    po[:, slot_idx, :], lhsT=wrep[:, :], rhs=g.qt[:, c, :],
                    start=True, stop=True, skip_group_check=True,
                )

            def emit_evac(gidx, c0, po, evac_on_act):
                g = glist[gidx]
                if g.outsb is None:
                    g.outsb = osb_pool.tile(
                        [128, n_iters, 256], bf16, tag="osb"
                    )
                dst = g.outsb[:, c0 : c0 + 2, :]
                if evac_on_act:
                    nc.scalar.activation(dst, po, func=Copy, scale=1.0)
                else:
                    nc.vector.tensor_copy(dst, po)
                half = n_iters * 256 // 2
                if c0 + 1 == n_iters // 2 - 1:
                    nc.gpsimd.dma_start(
                        out=out_d[g.b, g.gi, :, 0:half],
                        in_=g.outsb[:, 0 : n_iters // 2, :],
                    )
                elif c0 + 1 == n_iters - 1:
                    nc.gpsimd.dma_start(
                        out=out_d[g.b, g.gi, :, half : 2 * half],
                        in_=g.outsb[:, n_iters // 2 : n_iters, :],
                    )
                    g.outsb = None
                    g.qt = None
                    g.kv = None

            # ---------------- schedule -----------------------------------
            # chain slot s (0..191): wave w = s//64, r = s%64, c = r//WAVE,
            #   gp = r%WAVE, group g = w*WAVE+gp.
            # act item (g, tb): 4 DR matmuls at slots w*64+16*tb+gp-LAG ...
            #   +3, cast at +4.
            # group kv DMA one wave ahead (spread), qt half a wave ahead.
            events = {}

            def at(slot, fn, *args):
                events.setdefault(slot, []).append((fn, args))

            n_slots = nwaves * slots_per_wave
            for w in range(nwaves):
                for gp in range(WAVE):
                    gidx = w * WAVE + gp
                    if w == 0:
                        # wave 0: all kv h0 first, then qt h0, then h1s, so
                        # the first acts/outs gate on the least DMA bytes
                        at(-40 + 2 * gp, kv_half, gidx, 0)
                        at(-30 + 2 * gp, qt_half, gidx, 0)
                        at(-22 + 3 * gp, kv_half, gidx, 1)
                        at(-10 + 3 * gp, qt_half, gidx, 1)
                    else:
                        at(w * 64 - 72 + 10 * gp, kv_half, gidx, 0)
                        at(w * 64 - 70 + 10 * gp, kv_half, gidx, 1)
                        at(w * 64 - 67 + 10 * gp, qt_half, gidx, 0)
                        at(w * 64 - 65 + 10 * gp, qt_half, gidx, 1)
                    for tb in range(n_tb):
                        t0 = w * 64 + 16 * tb + 4 * gp - LAG
                        for u in range(CB):
                            at(t0 + u, emit_act, gidx, tb, u)
                        at(t0 + CB, emit_cast, gidx, tb)

            # pending out-pairs: (gidx, c0, w0, w1)
            pend = []
            prev_w = [None] * ngroups
            evac_flip = [0]

            lo = min(events)
            for s in range(lo, n_slots + 3):
                for fn, args in events.get(s, ()):
                    fn(*args)
                # interleave within the slot so no two consecutive matmuls
                # target the same PSUM bank, and DVE sees stt BEFORE evac:
                #   out(c0) [pout] .. seed [pg] .. out(c0+1) [pout]
                #   .. grad [pg] + stt .. evac
                po_info = None
                if pend and s >= 6:  # wave-0 warmup: let qt DMA land first
                    gq, c0q, w0q, w1q = pend.pop(0)
                    evac_flip[0] = (evac_flip[0] + 1) % 3
                    on_act = evac_flip[0] != 0  # 2/3 Act, 1/3 DVE
                    po = pout_pool.tile([128, 2, 256], f32, tag="po")
                    emit_out_mm(gq, c0q, w0q, po, 0)
                    po_info = (gq, c0q, w1q, po, on_act)
                in_chain = 0 <= s < n_slots
                if in_chain:
                    w, r = divmod(s, slots_per_wave)
                    c, gp = divmod(r, WAVE)
                    gidx = w * WAVE + gp
                    g = glist[gidx]
                    pg = chain_seed(g, c)
                if po_info is not None:
                    emit_out_mm(po_info[0], po_info[1] + 1, po_info[2],
                                po_info[3], 1)
                if in_chain:
                    chain_grad(g, c, pg)
                    if c % 2 == 1:
                        pend.append((gidx, c - 1, prev_w[gidx], g.wrep))
                    else:
                        prev_w[gidx] = g.wrep
                if po_info is not None:
                    emit_evac(po_info[0], po_info[1], po_info[3], po_info[4])
            while pend:
                gq, c0q, w0q, w1q = pend.pop(0)
                evac_flip[0] = (evac_flip[0] + 1) % 3
                po = pout_pool.tile([128, 2, 256], f32, tag="po")
                emit_out_mm(gq, c0q, w0q, po, 0)
                emit_out_mm(gq, c0q + 1, w1q, po, 1)
                emit_evac(gq, c0q, po, evac_flip[0] != 0)

    _split_excess_waits(nc)
    return nc


def _get_nc():
    if "nc" not in _CACHE:
        _CACHE["nc"] = _build()
    return _CACHE["nc"]


def _host_prep(q, k, v):
    """Host re-layout (token t = c*256 + j*128 + p)."""
    import ml_dtypes

    bf = ml_dtypes.bfloat16
    f8 = ml_dtypes.float8_e4m3
    Bq, Hq, Nq, Dq = q.shape
    hg = Hq // 2
    ni = Nq // 256
    # kv: [b, g, p, c, j, (k0|k1|v0|v1), d]
    k7 = k.reshape(Bq, hg, 2, ni, 2, 128, Dq)
    v7 = (-v).reshape(Bq, hg, 2, ni, 2, 128, Dq)
    kv = np.stack(
        [k7[:, :, 0], k7[:, :, 1], v7[:, :, 0], v7[:, :, 1]], axis=5
    )  # [b, g, c, j, p, 4, d]
    kv = np.ascontiguousarray(
        kv.transpose(0, 1, 4, 2, 3, 5, 6).reshape(Bq, hg, 128, ni * 2 * 4 * Dq)
    ).astype(f8)
    # qt: [b, g, hpair*64+d, c, t]
    q6 = q.reshape(Bq, hg, 2, ni, 256, Dq)
    qt = np.ascontiguousarray(
        q6.transpose(0, 1, 2, 5, 3, 4).reshape(Bq, hg, 128, ni * 256)
    ).astype(bf)
    return kv, qt


def _host_unshuffle(out_host):
    """[b, g, hpair*64+e, c*256+t] bf16 -> (B, N, H*64) f32."""
    Bq, hgq, _, w = out_host.shape
    ni = w // 256
    o6 = np.asarray(out_host, dtype=np.float32).reshape(
        Bq, hgq, 2, 64, ni, 256
    )
    # [b, g, hp, e, c, t] -> [b, c, t, g, hp, e]
    return np.ascontiguousarray(
        o6.transpose(0, 4, 5, 1, 2, 3).reshape(Bq, ni * 256, hgq * 2 * 64)
    )


def kernel(q, k, v, W_init, training=0, return_aux=0, **_unused):
    import ml_dtypes
    from concourse.bass_utils import run_bass_kernel_spmd

    q = np.asarray(q, dtype=np.float32)
    k = np.asarray(k, dtype=np.float32)
    v = np.asarray(v, dtype=np.float32)
    W_init = np.ascontiguousarray(np.asarray(W_init, dtype=np.float32))

    kv, qt = _host_prep(q, k, v)
    wbd = np.zeros((HG, 128, 128), dtype=np.float32)
    wbd[:, 0:64, 0:64] = W_init[0::2]
    wbd[:, 64:128, 64:128] = W_init[1::2]
    wbd = np.ascontiguousarray(
        wbd.transpose(1, 0, 2).reshape(128, HG * 128)
    )
    ident = np.eye(128, dtype=ml_dtypes.bfloat16)

    nc = _get_nc()
    in_maps = []
    for i in range(NCORES):
        sl = slice(i * NB, (i + 1) * NB)
        in_maps.append(
            {"qt": qt[sl], "kv": kv[sl], "W12bd": wbd, "ident": ident}
        )

    trace = bool(int(os.environ.get("BASS_KERNEL_TRACE", "0")))
    res = run_bass_kernel_spmd(
        nc, in_maps, core_ids=list(range(NCORES)), trace=trace
    )
    _CACHE["last_results"] = res
    out_host = np.concatenate(
        [np.asarray(res.results[i]["out"]) for i in range(NCORES)], axis=0
    )
    return _host_unshuffle(out_host)


if __name__ == "__main__":
    rng = np.random.default_rng(0)
    q = rng.standard_normal((B, H, N, D), dtype=np.float32)
    k = rng.standard_normal((B, H, N, D), dtype=np.float32)
    v = rng.standard_normal((B, H, N, D), dtype=np.float32)
    W = (rng.standard_normal((H, D, D)) * D**-0.5).astype(np.float32)
    out = kernel(q, k, v, W)
    print("kernel ran, out shape:", out.shape)
